# revision 1
# baseline (speedup 1.0000x reference)
"""Trainium2 Bass kernel for nn_Decoder (2-layer LSTM decoder + vocab head).

Computation (matches reference.py):
  embeds = emb[sentence]                      [B, T, E]
  x = concat(features, embeds[:, :-1])        [B, T, E]
  h0 = LSTM0(x), h1 = LSTM1(h0)               [B, T, H]
  out = (h1 @ fc_W.T + fc_b).transpose(0,2,1) [B, V, T]

Sharding (8 NeuronCores, SPMD, no collectives):
  - The LSTM is replicated on all cores: it is sequential in T and its
    per-step matmul cost is weight-ingestion bound (the whole W_hh must
    stream through the PE array every step), so batch-splitting would not
    reduce wall time while vocab-splitting the head needs h on every core.
  - The fc/vocab dimension is sharded 8 ways: 4000 rows per core (padded
    to 4096).  Each core writes its logits slice in [V_loc, B, T] layout
    (per-partition-contiguous 2 KB DMA runs; the problem's [B, V, T]
    layout would force 128 B descriptors, ~2.3x slower on the DMA path).
    The host unshard concatenates the slices and transposes to [B, V, T].

Device layout ("k-space"): every tensor entering a matmul keeps the
contraction dim on partitions:  X[p, kc, ...] == X_full[kc*128+p, ...].
Gate chunks land g-on-partitions, so the LSTM state (c, h) stays k-aligned
and feeds the next step's stationary operand without any transpose.

Schedule: the layers and projections are software-pipelined per step t:
  xp0 chunk | rec0(t) | xp1 chunk (every 4 steps) | rec1(t-4)
so each layer's post-matmul ACT/DVE chain hides under the other streams'
matmuls.  Input projections live in small SBUF ring buffers (bias folded
in) and are added to the gate PSUM with an identity-weight matmul, letting
ScalarE apply the nonlinearities straight out of PSUM.  All matmul
operands are bf16 (f32 PSUM accumulation); measured end-to-end rel. error
vs the f32 reference is ~3.5e-3.

Measured on trn2 (8 cores): ~474 us HW exec; LSTM ~310 us (PE-bound at
~52 ns per LDWEIGHTS+MATMUL pair), fc ~113 us (PE/DMA-balanced).

Environment note: this walrus build rejects >1 embedded sync wait per
instruction; _split_waits_json() rewrites the serialized BIR, hoisting
excess waits onto same-engine NoOp carriers (identical semantics).
"""

import numpy as np
import ml_dtypes

# ---------------------------------------------------------------------------
# Workaround: this walrus build caps instructions at ONE embedded sync wait
# ("Too many sync wait commands" in setupSyncWait); Tile routinely attaches
# several.  Post-process the serialized BIR: hoist excess waits of every
# instruction onto same-engine NoOp carriers inserted immediately before it.
# Semantics are identical (all waits still complete before the instruction
# executes on its engine).
# ---------------------------------------------------------------------------
import orjson
import concourse.tile as tile

_MAXW = 1


def _split_waits_json(b: bytes) -> bytes:
    d = orjson.loads(b)
    for f in d["functions"]:
        for blk in f["blocks"]:
            out = []
            for inst in blk["instructions"]:
                si = inst.get("sync_info")
                if si:
                    w = si.get("on_wait") or []
                    if len(w) > _MAXW:
                        for i, wt in enumerate(w[:-_MAXW]):
                            out.append(
                                {
                                    "debug": inst.get("debug", 0),
                                    "engine": inst["engine"],
                                    "ins": [],
                                    "outs": [],
                                    "name": f"{inst['name']}-hw{i}",
                                    "opcode": "NoOp",
                                    "sync_info": {"on_update": [], "on_wait": [wt]},
                                }
                            )
                        si["on_wait"] = w[-_MAXW:]
                out.append(inst)
            blk["instructions"] = out
    return orjson.dumps(d)


def _patch_serialization(nc):
    orig = nc.to_json_bytes
    nc.to_json_bytes = lambda: _split_waits_json(orig())
    return nc


import concourse.bass as bass
import concourse.mybir as mybir
from concourse.bass import ts, ds
from concourse.bass_utils import run_bass_kernel_spmd

F32 = mybir.dt.float32
BF16 = mybir.dt.bfloat16
AF = mybir.ActivationFunctionType
BF16_NP = ml_dtypes.bfloat16

E, H, V, B, T = 512, 512, 32000, 64, 32
G = 4 * H                    # 2048 gate rows per layer
KC = 4                       # 512 = 4 k-chunks of 128
NCORES = 8
VPAD = 4096                  # per-core vocab slice, padded from 4000
NTOK = B * T                 # 2048
LAG = 4                      # rec1 runs LAG steps behind rec0


def _build_nc():
    nc = bass.Bass()

    xT_d = nc.dram_tensor("xT", [128, KC, NTOK], BF16, kind="ExternalInput")
    wih0_d = nc.dram_tensor("wih0T", [128, KC, G], BF16, kind="ExternalInput")
    whh0_d = nc.dram_tensor("whh0T", [128, KC, G], BF16, kind="ExternalInput")
    wih1_d = nc.dram_tensor("wih1T", [128, KC, G], BF16, kind="ExternalInput")
    whh1_d = nc.dram_tensor("whh1T", [128, KC, G], BF16, kind="ExternalInput")
    b0_d = nc.dram_tensor("b0", [128, 16], F32, kind="ExternalInput")
    b1_d = nc.dram_tensor("b1", [128, 16], F32, kind="ExternalInput")
    ident_d = nc.dram_tensor("ident", [128, 128], BF16, kind="ExternalInput")
    fcw_d = nc.dram_tensor("fcwT", [128, KC, VPAD], BF16, kind="ExternalInput")
    fcb_d = nc.dram_tensor("fcb", [128, VPAD // 128], F32, kind="ExternalInput")
    out_d = nc.dram_tensor("out", [VPAD, B, T], F32, kind="ExternalOutput")

    with tile.TileContext(nc) as tc:
        with (
            tc.tile_pool(name="consts", bufs=1) as consts,
            tc.tile_pool(name="state", bufs=1) as state,
            tc.tile_pool(name="ps_gates", bufs=2, space="PSUM") as ps_gates,
            tc.tile_pool(name="ps_big", bufs=4, space="PSUM") as ps_big,
        ):
            # ---- small constants (DMAs emitted later, after weight loads) ----
            b0_sb = consts.tile([128, 16], F32, tag="b0")
            b1_sb = consts.tile([128, 16], F32, tag="b1")
            fcb_sb = consts.tile([128, VPAD // 128], F32, tag="fcb")
            ident = consts.tile([128, 128], BF16, tag="ident")

            # ---- histories ----
            hist0 = consts.tile([128, KC, T, B], BF16, tag="hist0")   # t-major
            hist1t = consts.tile([128, KC, T, B], BF16, tag="hist1t")  # t-major (rec)
            hist1b = consts.tile([128, KC, B, T], BF16, tag="hist1b")  # b-major (fc)
            # SBUF rings for the bias-folded input projections (8 slabs each)
            xp0r = consts.tile([128, 12, 16, B], BF16, tag="xp0r")
            xp1r = consts.tile([128, 8, 16, B], BF16, tag="xp1r")

            # ---- per-layer state ----
            st = []
            for l in range(2):
                cT = state.tile([128, KC, B], F32, tag=f"cT{l}", name=f"cT{l}")
                gates = state.tile([128, 16, B], F32, tag=f"gates{l}", name=f"gates{l}")
                tmp1 = state.tile([128, KC, B], F32, tag=f"tmp1{l}", name=f"tmp1{l}")
                tmp2 = state.tile([128, KC, B], F32, tag=f"tmp2{l}", name=f"tmp2{l}")
                tanh_c = state.tile([128, KC, B], F32, tag=f"tanhc{l}", name=f"tanhc{l}")
                st.append(dict(cT=cT, gates=gates, tmp1=tmp1, tmp2=tmp2, tanh_c=tanh_c))

            def xp_chunk(w_sb, rhs_slice, bias_sb, ring, c):
                """xp chunk c = slabs 4c..4c+3 -> ring slots (4c) mod ringlen."""
                n0, ntoks = c * 4 * B, 4 * B
                s0 = (4 * c) % ring.shape[1]
                for g in range(16):
                    ps = ps_big.tile([128, 4, B], F32, tag="ps512")
                    for kc in range(KC):
                        nc.tensor.matmul(
                            ps,
                            w_sb[:, kc, ts(g, 128)],
                            rhs_slice(kc, n0, ntoks),
                            start=(kc == 0),
                            stop=(kc == KC - 1),
                        )
                    nc.scalar.activation(
                        out=ring[:, ds(s0, 4), g, :], in_=ps, func=AF.Identity,
                        bias=bias_sb[:, g : g + 1], scale=1.0,
                    )

            def rec_step(l, t, whh_sb, ring, hist_rd, hist_wr):
                s = st[l]
                xsl = ring[:, (4 * (t // 4) % ring.shape[1]) + t % 4, :, :]  # bias folded
                ps0 = ps_gates.tile([128, 8, B], F32, tag="ps0")
                ps1 = ps_gates.tile([128, 8, B], F32, tag="ps1")
                for half, ps in ((0, ps0), (1, ps1)):
                    if t > 0:
                        for j in range(8):
                            gc = half * 8 + j
                            for kc in range(KC):
                                nc.tensor.matmul(
                                    ps[:, j, :],
                                    whh_sb[:, kc, ts(gc, 128)],
                                    hist_rd(kc, t - 1),
                                    start=(j == 0 and kc == 0),
                                    stop=False,
                                    skip_group_check=True,
                                )
                    # fold xp into the PSUM group via identity weights
                    nc.tensor.matmul(
                        ps,
                        ident,
                        xsl[:, ts(half, 8), :],
                        start=(t == 0),
                        stop=True,
                        skip_group_check=True,
                    )
                g = s["gates"]
                nc.scalar.activation(g[:, 0:8, :], ps0, func=AF.Sigmoid)
                nc.scalar.activation(g[:, 8:12, :], ps1[:, 0:4, :], func=AF.Tanh)
                nc.scalar.activation(g[:, 12:16, :], ps1[:, 4:8, :], func=AF.Sigmoid)
                if t == 0:
                    nc.vector.tensor_mul(s["cT"], g[:, 0:4, :], g[:, 8:12, :])
                else:
                    nc.vector.tensor_mul(s["tmp1"], g[:, 0:4, :], g[:, 8:12, :])
                    nc.vector.tensor_mul(s["tmp2"], g[:, 4:8, :], s["cT"])
                    nc.vector.tensor_add(s["cT"], s["tmp1"], s["tmp2"])
                nc.scalar.activation(s["tanh_c"], s["cT"], func=AF.Tanh)
                for wr in hist_wr(t):
                    nc.vector.tensor_mul(wr, g[:, 12:16, :], s["tanh_c"])

            with tc.tile_pool(name="wpool", bufs=1) as wpool:
                whh0_sb = wpool.tile([128, KC, G], BF16, tag="whh0")
                nc.gpsimd.dma_start(out=whh0_sb, in_=whh0_d[:])
                wih1_sb = wpool.tile([128, KC, G], BF16, tag="wih1")
                whh1_sb = wpool.tile([128, KC, G], BF16, tag="whh1")

                rec0 = dict(
                    whh_sb=whh0_sb,
                    ring=xp0r,
                    hist_rd=lambda kc, t: hist0[:, kc, t, :],
                    hist_wr=lambda t: [hist0[:, :, t, :]],
                )
                rec1 = dict(
                    whh_sb=whh1_sb,
                    ring=xp1r,
                    hist_rd=lambda kc, t: hist1t[:, kc, t, :],
                    hist_wr=lambda t: [hist1t[:, :, t, :], hist1b[:, :, :, t]],
                )

                with tc.tile_pool(name="inpool", bufs=1) as inpool:
                    xT_sb = inpool.tile([128, KC, NTOK], BF16, tag="xT")
                    wih0_sb = inpool.tile([128, KC, G], BF16, tag="wih0")
                    # split loads so the first xp0 chunk's pieces land first
                    for piece in range(4):
                        nc.sync.dma_start(
                            out=xT_sb[:, :, ts(piece, 512)],
                            in_=xT_d[:, :, ts(piece, 512)],
                        )
                        nc.scalar.dma_start(
                            out=wih0_sb[:, :, ts(piece, 512)],
                            in_=wih0_d[:, :, ts(piece, 512)],
                        )
                    nc.scalar.dma_start(out=b0_sb, in_=b0_d[:])
                    nc.scalar.dma_start(out=b1_sb, in_=b1_d[:])
                    nc.scalar.dma_start(out=fcb_sb, in_=fcb_d[:])
                    nc.scalar.dma_start(out=ident, in_=ident_d[:])

                    xp0_rhs = lambda kc, n0, nt: xT_sb[:, kc, ds(n0, nt)]
                    xp1_rhs = lambda kc, n0, nt: hist0[:, kc, ds(n0 // B, nt // B), :]

                    # xp0 chunk 0, then the software-pipelined steady state
                    xp_chunk(wih0_sb, xp0_rhs, b0_sb, xp0r, 0)
                    for t in range(T):
                        if t == 0:
                            nc.gpsimd.dma_start(out=wih1_sb, in_=wih1_d[:])
                        if t == 1:
                            nc.gpsimd.dma_start(out=whh1_sb, in_=whh1_d[:])
                        if t in (0, 1):
                            xp_chunk(wih0_sb, xp0_rhs, b0_sb, xp0r, t + 1)
                        elif t % 4 == 0 and 3 <= t // 4 + 2 <= 7:
                            xp_chunk(wih0_sb, xp0_rhs, b0_sb, xp0r, t // 4 + 2)
                        rec_step(0, t, **rec0)
                        if t % 4 == 3:
                            xp_chunk(wih1_sb, xp1_rhs, b1_sb, xp1r, t // 4)
                        if t >= LAG:
                            rec_step(1, t - LAG, **rec1)
                for s_ in range(T - LAG, T):
                    rec_step(1, s_, **rec1)

            # ================= fc =================
            with (
                tc.tile_pool(name="fcpool", bufs=1) as fcpool,
                tc.tile_pool(name="fcstage", bufs=6) as fcstage,
            ):
                fcw_sb = fcpool.tile([128, KC, VPAD], BF16, tag="fcw")
                nc.gpsimd.dma_start(out=fcw_sb, in_=fcw_d[:])
                dma_engines = [nc.sync, nc.scalar]
                for v in range(VPAD // 128):
                    for n in range(4):
                        ps = ps_big.tile([128, 16, T], F32, tag="ps512")
                        for kc in range(KC):
                            nc.tensor.matmul(
                                ps,
                                fcw_sb[:, kc, ts(v, 128)],
                                hist1b[:, kc, ts(n, 16), :],
                                start=(kc == 0),
                                stop=(kc == KC - 1),
                            )
                        ot = fcstage.tile([128, 16, T], F32, tag="ot")
                        if (v * 4 + n) % 2 == 0:
                            nc.scalar.activation(
                                out=ot, in_=ps, func=AF.Identity,
                                bias=fcb_sb[:, v : v + 1], scale=1.0,
                            )
                        else:
                            nc.vector.tensor_scalar_add(ot, ps, fcb_sb[:, v : v + 1])
                        eng = dma_engines[(v * 4 + n) % len(dma_engines)]
                        eng.dma_start(
                            out=out_d[ts(v, 128), ds(16 * n, 16), :],
                            in_=ot,
                        )
    return _patch_serialization(nc)


def _to_k128(W, dtype):
    """W [out_dim, K] -> [128, K//128, out_dim] with result[p,kc,g]=W[g,kc*128+p]."""
    K = W.shape[1]
    return np.ascontiguousarray(
        W.T.reshape(K // 128, 128, -1).transpose(1, 0, 2)
    ).astype(dtype)


_NC_CACHE = None
RUN_KWARGS = {}
LAST_RESULT = None


def kernel(
    sentence,
    features,
    lengths,
    emb,
    W_ih0,
    W_hh0,
    b_ih0,
    b_hh0,
    W_ih1,
    W_hh1,
    b_ih1,
    b_hh1,
    fc_W,
    fc_b,
):
    global _NC_CACHE, LAST_RESULT
    sentence = np.asarray(sentence).astype(np.int64)
    features = np.asarray(features, dtype=np.float32)
    emb = np.asarray(emb, dtype=np.float32)

    # embedding gather + teacher forcing shift (host; pure data movement)
    embeds = emb[sentence[:, : T - 1]]                      # [B, T-1, E]
    x = np.concatenate([features[:, None, :], embeds], axis=1)  # [B, T, E]
    # token-major [k, tok] with tok = t*B + b
    xT = np.ascontiguousarray(x.transpose(2, 1, 0).reshape(E, NTOK))
    xT_p = np.ascontiguousarray(
        xT.reshape(KC, 128, NTOK).transpose(1, 0, 2)
    ).astype(BF16_NP)

    wih0 = _to_k128(np.asarray(W_ih0, np.float32), BF16_NP)
    whh0 = _to_k128(np.asarray(W_hh0, np.float32), BF16_NP)
    wih1 = _to_k128(np.asarray(W_ih1, np.float32), BF16_NP)
    whh1 = _to_k128(np.asarray(W_hh1, np.float32), BF16_NP)
    b0 = np.ascontiguousarray(
        (np.asarray(b_ih0, np.float32) + np.asarray(b_hh0, np.float32))
        .reshape(16, 128)
        .T
    )
    b1 = np.ascontiguousarray(
        (np.asarray(b_ih1, np.float32) + np.asarray(b_hh1, np.float32))
        .reshape(16, 128)
        .T
    )

    fc_W = np.asarray(fc_W, np.float32)
    fc_b = np.asarray(fc_b, np.float32)
    vloc = V // NCORES  # 4000 real rows per core, padded to VPAD

    common = {
        "xT": xT_p,
        "wih0T": wih0,
        "whh0T": whh0,
        "wih1T": wih1,
        "whh1T": whh1,
        "b0": b0,
        "b1": b1,
        "ident": np.eye(128, dtype=BF16_NP),
    }
    in_maps = []
    for c in range(NCORES):
        wslice = np.zeros((VPAD, E), np.float32)
        wslice[:vloc] = fc_W[c * vloc : (c + 1) * vloc]
        bslice = np.zeros(VPAD, np.float32)
        bslice[:vloc] = fc_b[c * vloc : (c + 1) * vloc]
        wc = _to_k128(wslice, BF16_NP)
        bc = np.ascontiguousarray(bslice.reshape(VPAD // 128, 128).T)
        in_maps.append({**common, "fcwT": wc, "fcb": bc})

    if _NC_CACHE is None:
        _NC_CACHE = _build_nc()

    res = run_bass_kernel_spmd(
        _NC_CACHE, in_maps, core_ids=list(range(NCORES)), **RUN_KWARGS
    )
    LAST_RESULT = res
    full = np.concatenate(
        [res.results[c]["out"][:vloc] for c in range(NCORES)], axis=0
    )  # [V, B, T]
    return np.ascontiguousarray(full.transpose(1, 0, 2))

